# revision 1
# baseline (speedup 1.0000x reference)
"""Luong 'general' attention kernel for TRN2, data-parallel over batch on 8 cores.

Reference computes:
    proj[l,b,g]   = sum_h enc[l,b,h] * W[g,h] + bias[g]
    energies[b,l] = sum_g hidden[b,g] * proj[l,b,g]
    out           = softmax_l(energies)[:, None, :]

Algebraic restructure (exact):
    energies[b,l] = sum_h v[b,h] * enc[l,b,h] + c[b],   v = hidden @ W
and c[b] = hidden[b]·bias is constant over l, so it cancels in softmax.
This reduces the work from O(L*B*H*H) to O(B*H*H + L*B*H): the kernel is
bound by streaming enc (32 MB/core) from HBM, not by compute.

Per-core layout (B sharded 8 ways, bb = 8 batches/core):
    encT[hc, h_in, bb, l]  -- host-transposed so H is on partitions and each
                              SBUF partition row is 32 KB contiguous in DRAM
    hT[g_in, gc, bb]       -- host-transposed hidden
    W[g, h]                -- replicated
Device:
    vT[h, bb]   = sum_g W[g,h] hidden[bb,g]        (PE, PSUM-accumulated)
    vTpad       -- per (hc,bb) a [128,8] weight with only column bb nonzero,
                   so all 8 bb accumulate into one [8,1024] PSUM energies
                   tile without polluting each other's rows
    E[bb, l]    = sum_hc matmul(vTpad, encT-tile)  (PSUM accumulation)
    softmax     -- reduce_max(negate) -> exp(x-max) w/ fused row-sum -> 1/sum
"""

import numpy as np

import concourse.bacc as bacc
import concourse.mybir as mybir
import concourse.tile as tile
from concourse.bass_utils import run_bass_kernel_spmd

B, L, H = 64, 1024, 1024
N_CORES = 8
BB = B // N_CORES  # batches per core
P = 128            # partitions
HC = H // P        # h chunks
GC = H // P        # g chunks
F32 = mybir.dt.float32

_CACHE = {}


def _build_nc():
    nc = bacc.Bacc(
        "TRN2", target_bir_lowering=False, debug=False, num_devices=N_CORES
    )

    encT_d = nc.dram_tensor("encT", [HC, P, BB, L], F32, kind="ExternalInput")
    w_d = nc.dram_tensor("w", [H, H], F32, kind="ExternalInput")
    hT_d = nc.dram_tensor("hT", [P, GC, BB], F32, kind="ExternalInput")
    out_d = nc.dram_tensor("out", [BB, L], F32, kind="ExternalOutput")

    with tile.TileContext(nc) as tc:
        with (
            tc.tile_pool(name="wpool", bufs=1) as wpool,
            tc.tile_pool(name="encpool", bufs=3) as encpool,
            tc.tile_pool(name="small", bufs=1) as small,
            tc.tile_pool(name="psum", bufs=1, space="PSUM") as psum,
        ):
            hT_sb = small.tile([P, GC, BB], F32)
            nc.sync.dma_start(out=hT_sb[:], in_=hT_d[:])

            w_tiles = []
            for gc in range(GC):
                wt = wpool.tile([P, H], F32, tag=f"w{gc}", name=f"w_sb{gc}")
                nc.sync.dma_start(out=wt[:], in_=w_d[gc * P : (gc + 1) * P, :])
                w_tiles.append(wt)

            # vT[h, bb] accumulated over g-chunks; layout [h_in, hc, bb]
            vT_ps = psum.tile([P, HC, BB], F32)
            for hc in range(HC):
                for gc in range(GC):
                    nc.tensor.matmul(
                        vT_ps[:, hc, :],
                        w_tiles[gc][:, hc * P : (hc + 1) * P],
                        hT_sb[:, gc, :],
                        start=(gc == 0),
                        stop=(gc == GC - 1),
                    )

            # zero-padded weights: vTpad[:, hc, bb*BB + m] = vT * (m == bb)
            vTpad = small.tile([P, HC, BB * BB], F32)
            nc.vector.memset(vTpad[:], 0.0)
            for hc in range(HC):
                nc.vector.tensor_copy(
                    vTpad[:, hc, 0 : BB * BB : BB + 1],
                    vT_ps[:, hc, :],
                )

            # energies accumulate into one [BB, L] PSUM tile
            E_ps = psum.tile([BB, L], F32)
            NL = 512  # fp32 moving-operand max per matmul
            for hc in range(HC):
                enc_sb = encpool.tile(
                    [P, BB, L], F32, tag="enc", name=f"enc_sb{hc}"
                )
                nc.sync.dma_start(out=enc_sb[:], in_=encT_d[hc])
                for bb in range(BB):
                    for lt in range(L // NL):
                        nc.tensor.matmul(
                            E_ps[:, lt * NL : (lt + 1) * NL],
                            vTpad[:, hc, bb * BB : (bb + 1) * BB],
                            enc_sb[:, bb, lt * NL : (lt + 1) * NL],
                            start=(hc == 0 and bb == 0),
                            stop=(hc == HC - 1 and bb == BB - 1),
                        )

            # softmax over l (free dim), rows are batches
            negmax = small.tile([BB, 1], F32)
            nc.vector.reduce_max(
                negmax[:], E_ps[:], axis=mybir.AxisListType.X, negate=True
            )
            p_sb = small.tile([BB, L], F32)
            esum = small.tile([BB, 1], F32)
            nc.scalar.activation(
                p_sb[:],
                E_ps[:],
                mybir.ActivationFunctionType.Exp,
                bias=negmax[:],
                scale=1.0,
                accum_out=esum[:],
            )
            rec = small.tile([BB, 1], F32)
            nc.vector.reciprocal(rec[:], esum[:])
            o_sb = small.tile([BB, L], F32)
            nc.vector.tensor_scalar_mul(o_sb[:], p_sb[:], rec[:])
            nc.sync.dma_start(out=out_d[:], in_=o_sb[:])

    nc.compile()
    return nc


def _get_nc():
    if "nc" not in _CACHE:
        _CACHE["nc"] = _build_nc()
    return _CACHE["nc"]


def kernel(hidden, encoder_outputs, W, b):
    hidden = np.asarray(hidden, dtype=np.float32)
    enc = np.asarray(encoder_outputs, dtype=np.float32)
    W = np.asarray(W, dtype=np.float32)

    nc = _get_nc()
    w_full = np.ascontiguousarray(W)
    in_maps = []
    for c in range(N_CORES):
        sl = slice(c * BB, (c + 1) * BB)
        # [L, BB, H] -> [H, BB, L] -> [HC, P, BB, L]
        encT = np.ascontiguousarray(
            enc[:, sl, :].transpose(2, 1, 0)
        ).reshape(HC, P, BB, L)
        # [BB, H] -> [H, BB] -> [GC, P, BB] -> [P, GC, BB]
        hT = np.ascontiguousarray(
            hidden[0, sl, :].T.reshape(GC, P, BB).transpose(1, 0, 2)
        )
        in_maps.append({"encT": encT, "w": w_full, "hT": hT})

    res = run_bass_kernel_spmd(nc, in_maps, list(range(N_CORES))).results
    out = np.concatenate([res[c]["out"] for c in range(N_CORES)], axis=0)
    return out[:, None, :]


# revision 13
# speedup vs baseline: 1.1570x; 1.1570x over previous
"""Luong 'general' attention kernel for TRN2, data-parallel over batch on 8 cores.

Reference computes:
    proj[l,b,g]   = sum_h enc[l,b,h] * W[g,h] + bias[g]
    energies[b,l] = sum_g hidden[b,g] * proj[l,b,g]
    out           = softmax_l(energies)[:, None, :]

Algebraic restructure (exact):
    energies[b,l] = sum_h v[b,h] * enc[l,b,h] + c[b],   v = hidden @ W
and c[b] = hidden[b]·bias is constant over l, so it cancels in softmax.
This reduces the work from O(L*B*H*H) to O(B*H*H + L*B*H): the kernel is
bound by streaming enc (32 MB/core) from HBM, not by compute.

Per-core layout (B sharded 8 ways, bb = 8 batches/core):
    encT[hc, h_in, bb, l]  -- host-transposed so H is on partitions and each
                              SBUF partition row is 32 KB contiguous in DRAM
    hT[g_in, gc, bb]       -- host-transposed hidden
    W[g, h]                -- replicated
Device:
    vT[h, bb]   = sum_g W[g,h] hidden[bb,g]        (PE, PSUM-accumulated)
    vTpad       -- per (hc,bb) a [128,8] weight with only column bb nonzero,
                   so all 8 bb accumulate into one [8,1024] PSUM energies
                   tile without polluting each other's rows
    E[bb, l]    = sum_hc matmul(vTpad, encT-tile)  (PSUM accumulation)
    softmax     -- reduce_max(negate) -> exp(x-max) w/ fused row-sum -> 1/sum
"""

import numpy as np

import concourse.bacc as bacc
import concourse.mybir as mybir
import concourse.tile as tile
from concourse.bass_utils import run_bass_kernel_spmd

B, L, H = 64, 1024, 1024
N_CORES = 8
BB = B // N_CORES  # batches per core
P = 128            # partitions
HC = H // P        # h chunks
GC = H // P        # g chunks
F32 = mybir.dt.float32
F32R = mybir.dt.float32r

_CACHE = {}


def _tf32_round(a):
    """Round fp32 to tf32/float32r (10-bit mantissa) with round-to-nearest-
    even, so the bits we DMA are already valid f32r values."""
    u = a.view(np.uint32)
    rounded = (u + np.uint32(0x0FFF) + ((u >> np.uint32(13)) & np.uint32(1))) & np.uint32(0xFFFFE000)
    return rounded.view(np.float32)


def _build_nc():
    nc = bacc.Bacc(
        "TRN2", target_bir_lowering=False, debug=False, num_devices=N_CORES
    )

    encT_d = nc.dram_tensor("encT", [HC, P, BB, L], F32R, kind="ExternalInput")
    w_d = nc.dram_tensor("w", [H, H], F32R, kind="ExternalInput")
    hT_d = nc.dram_tensor("hT", [P, GC, BB], F32R, kind="ExternalInput")
    # zero template for vTpad (MEMSET can't encode f32r)
    zpad_d = nc.dram_tensor("zpad", [P, HC * BB * BB], F32R, kind="ExternalInput")
    out_d = nc.dram_tensor("out", [BB, L], F32, kind="ExternalOutput")

    with tile.TileContext(nc) as tc:
        with (
            tc.tile_pool(name="wpool", bufs=1) as wpool,
            tc.tile_pool(name="encpool", bufs=3) as encpool,
            tc.tile_pool(name="small", bufs=1) as small,
            tc.tile_pool(name="psum", bufs=1, space="PSUM") as psum,
        ):
            # W + hidden on the SWDGE ring so the two HWDGE rings are
            # dedicated to streaming enc
            hT_sb = small.tile([P, GC, BB], F32R)
            nc.gpsimd.dma_start(out=hT_sb[:], in_=hT_d[:])

            w_tiles = []
            for gc in range(GC):
                wt = wpool.tile([P, H], F32R, tag=f"w{gc}", name=f"w_sb{gc}")
                nc.gpsimd.dma_start(out=wt[:], in_=w_d[gc * P : (gc + 1) * P, :])
                w_tiles.append(wt)

            # vT[h, bb] accumulated over g-chunks; layout [h_in, hc, bb]
            vT_ps = psum.tile([P, HC, BB], F32)
            for hc in range(HC):
                for gc in range(GC):
                    nc.tensor.matmul(
                        vT_ps[:, hc, :],
                        w_tiles[gc][:, hc * P : (hc + 1) * P],
                        hT_sb[:, gc, :],
                        start=(gc == 0),
                        stop=(gc == GC - 1),
                    )

            # zero-padded weights: vTpad[:, hc, bb*BB + m] = vT * (m == bb)
            vTpad = small.tile([P, HC, BB * BB], F32R)
            nc.gpsimd.dma_start(out=vTpad[:], in_=zpad_d[:])
            for hc in range(HC):
                nc.vector.tensor_copy(
                    vTpad[:, hc, 0 : BB * BB : BB + 1],
                    vT_ps[:, hc, :],
                )

            # energies accumulate into one [BB, L] PSUM tile
            E_ps = psum.tile([BB, L], F32)
            NL = 512  # fp32 moving-operand max per matmul
            for hc in range(HC):
                enc_sb = encpool.tile(
                    [P, BB, L], F32R, tag="enc", name=f"enc_sb{hc}"
                )
                # alternate the two HWDGE rings (SP / ACT) so consecutive
                # 4MB tile loads stream concurrently instead of FIFO on one
                dma_eng = nc.sync if hc % 2 == 0 else nc.scalar
                dma_eng.dma_start(out=enc_sb[:], in_=encT_d[hc])
                for bb in range(BB):
                    for lt in range(L // NL):
                        nc.tensor.matmul(
                            E_ps[:, lt * NL : (lt + 1) * NL],
                            vTpad[:, hc, bb * BB : (bb + 1) * BB],
                            enc_sb[:, bb, lt * NL : (lt + 1) * NL],
                            start=(hc == 0 and bb == 0),
                            stop=(hc == HC - 1 and bb == BB - 1),
                        )

            # softmax over l (free dim), rows are batches
            negmax = small.tile([BB, 1], F32)
            nc.vector.reduce_max(
                negmax[:], E_ps[:], axis=mybir.AxisListType.X, negate=True
            )
            p_sb = small.tile([BB, L], F32)
            esum = small.tile([BB, 1], F32)
            nc.scalar.activation(
                p_sb[:],
                E_ps[:],
                mybir.ActivationFunctionType.Exp,
                bias=negmax[:],
                scale=1.0,
                accum_out=esum[:],
            )
            rec = small.tile([BB, 1], F32)
            nc.vector.reciprocal(rec[:], esum[:])
            o_sb = small.tile([BB, L], F32)
            nc.vector.tensor_scalar_mul(o_sb[:], p_sb[:], rec[:])
            nc.sync.dma_start(out=out_d[:], in_=o_sb[:])

    nc.compile()
    return nc


def _get_nc():
    if "nc" not in _CACHE:
        _CACHE["nc"] = _build_nc()
    return _CACHE["nc"]


def _make_in_maps(hidden, enc, W):
    w_full = _tf32_round(np.ascontiguousarray(W))
    in_maps = []
    for c in range(N_CORES):
        sl = slice(c * BB, (c + 1) * BB)
        # [L, BB, H] -> [H, BB, L] -> [HC, P, BB, L]
        encT = _tf32_round(
            np.ascontiguousarray(enc[:, sl, :].transpose(2, 1, 0)).reshape(
                HC, P, BB, L
            )
        )
        # [BB, H] -> [H, BB] -> [GC, P, BB] -> [P, GC, BB]
        hT = _tf32_round(
            np.ascontiguousarray(
                hidden[0, sl, :].T.reshape(GC, P, BB).transpose(1, 0, 2)
            )
        )
        in_maps.append(
            {
                "encT": encT,
                "w": w_full,
                "hT": hT,
                "zpad": np.zeros((P, HC * BB * BB), np.float32),
            }
        )
    return in_maps


def kernel(hidden, encoder_outputs, W, b):
    hidden = np.asarray(hidden, dtype=np.float32)
    enc = np.asarray(encoder_outputs, dtype=np.float32)
    W = np.asarray(W, dtype=np.float32)

    nc = _get_nc()
    in_maps = _make_in_maps(hidden, enc, W)
    res = run_bass_kernel_spmd(nc, in_maps, list(range(N_CORES))).results
    out = np.concatenate([res[c]["out"] for c in range(N_CORES)], axis=0)
    return out[:, None, :]


# revision 16
# speedup vs baseline: 1.2038x; 1.0405x over previous
"""Luong 'general' attention kernel for TRN2, data-parallel over batch on 8 cores.

Reference computes:
    proj[l,b,g]   = sum_h enc[l,b,h] * W[g,h] + bias[g]
    energies[b,l] = sum_g hidden[b,g] * proj[l,b,g]
    out           = softmax_l(energies)[:, None, :]

Algebraic restructure (exact):
    energies[b,l] = sum_h v[b,h] * enc[l,b,h] + c[b],   v = hidden @ W
and c[b] = hidden[b]·bias is constant over l, so it cancels in softmax.
This reduces the work from O(L*B*H*H) to O(B*H*H + L*B*H): the kernel is
bound by streaming enc (32 MB/core) from HBM, not by compute.

Precision strategy: fp32 matmuls cost 4 cycles/row on TensorE, so enc is
split on the host into bf16 hi + bf16 lo (e = e_hi + e_lo exactly to
~2^-17) and v is split on-device the same way. Energies are assembled
from three bf16 products, all PSUM-accumulated in fp32:
    rows 0-7  of A:  e_hi.v_hi + e_lo.v_hi      (two matmul streams)
    rows 8-15 of A:  e_hi.v_lo                  (packed in the e_hi stream
                                                 via a 16-col weight)
    E = A[0:8] + A[8:16]   (the dropped e_lo.v_lo term is ~2^-18 relative)
This keeps fp32-class accuracy at bf16 streaming rate (1 cycle/row).

Per-core layout (B sharded 8 ways, bb = 8 batches/core):
    ehi/elo[hc, h_in, bb, l] -- host-transposed so H is on partitions; each
                                SBUF partition row is 16 KB contiguous DRAM
    hT[g_in, gc, bb]         -- host-transposed hidden (fp32)
    W[g, h]                  -- replicated (fp32; v must be fp32-accurate)
DMA: W goes first, split across both HWDGE rings (SP + ACT), then enc
tiles alternate rings; small tensors ride the SWDGE ring.
"""

import numpy as np
import ml_dtypes

import concourse.bacc as bacc
import concourse.mybir as mybir
import concourse.tile as tile
from concourse.bass_utils import run_bass_kernel_spmd

B, L, H = 64, 1024, 1024
N_CORES = 8
BB = B // N_CORES  # batches per core
P = 128            # partitions
HC = H // P        # h chunks
GC = H // P        # g chunks
F32 = mybir.dt.float32
BF16 = mybir.dt.bfloat16
BF16NP = ml_dtypes.bfloat16

_CACHE = {}


def _build_nc():
    nc = bacc.Bacc(
        "TRN2", target_bir_lowering=False, debug=False, num_devices=N_CORES
    )

    ehi_d = nc.dram_tensor("ehi", [HC, P, BB, L], BF16, kind="ExternalInput")
    elo_d = nc.dram_tensor("elo", [HC, P, BB, L], BF16, kind="ExternalInput")
    w_d = nc.dram_tensor("w", [H, H], F32, kind="ExternalInput")
    hT_d = nc.dram_tensor("hT", [P, GC, BB], F32, kind="ExternalInput")
    out_d = nc.dram_tensor("out", [BB, L], F32, kind="ExternalOutput")

    with tile.TileContext(nc) as tc:
        with (
            tc.tile_pool(name="wpool", bufs=1) as wpool,
            tc.tile_pool(name="encpool", bufs=4) as encpool,
            tc.tile_pool(name="small", bufs=1) as small,
            tc.tile_pool(name="psum", bufs=1, space="PSUM") as psum,
        ):
            hT_sb = small.tile([P, GC, BB], F32)
            nc.gpsimd.dma_start(out=hT_sb[:], in_=hT_d[:])

            # W first, split across both HWDGE rings, so v is ready early
            w_tiles = []
            for gc in range(GC):
                wt = wpool.tile([P, H], F32, tag=f"w{gc}", name=f"w_sb{gc}")
                eng = nc.sync if gc % 2 == 0 else nc.scalar
                eng.dma_start(out=wt[:], in_=w_d[gc * P : (gc + 1) * P, :])
                w_tiles.append(wt)

            # vT[h, bb] = sum_g W[g,h] hidden[bb,g], accumulated over gc
            vT_ps = psum.tile([P, HC, BB], F32)
            for hc in range(HC):
                for gc in range(GC):
                    nc.tensor.matmul(
                        vT_ps[:, hc, :],
                        w_tiles[gc][:, hc * P : (hc + 1) * P],
                        hT_sb[:, gc, :],
                        start=(gc == 0),
                        stop=(gc == GC - 1),
                    )

            # split v into bf16 hi + lo on-device
            vhi = small.tile([P, HC, BB], BF16)
            nc.vector.tensor_copy(vhi[:], vT_ps[:])
            vlo = small.tile([P, HC, BB], BF16)
            nc.vector.tensor_sub(vlo[:], vT_ps[:], vhi[:])

            # packed diag weights: per (hc, bb) a [128, 16] block whose
            # col bb is v_hi and col 8+bb is v_lo, zeros elsewhere
            vpad = small.tile([P, HC, BB, 2 * BB], BF16)
            nc.vector.memset(vpad[:], 0.0)
            for hc in range(HC):
                blk = vpad[:, hc].rearrange("p a b -> p (a b)")
                nc.vector.tensor_copy(
                    blk[:, 0 : BB * 2 * BB : 2 * BB + 1], vhi[:, hc, :]
                )
                nc.vector.tensor_copy(
                    blk[:, BB : BB * 2 * BB : 2 * BB + 1], vlo[:, hc, :]
                )

            # A rows 0-7: e_hi.v_hi + e_lo.v_hi ; rows 8-15: e_hi.v_lo
            A_ps = psum.tile([2 * BB, L], F32)
            NL = 512  # one fp32 PSUM bank per matmul
            for hc in range(HC):
                ehi_sb = encpool.tile(
                    [P, BB, L], BF16, tag="ehi", name=f"ehi_sb{hc}"
                )
                elo_sb = encpool.tile(
                    [P, BB, L], BF16, tag="elo", name=f"elo_sb{hc}"
                )
                eng = nc.sync if hc % 2 == 0 else nc.scalar
                eng2 = nc.scalar if hc % 2 == 0 else nc.sync
                eng.dma_start(out=ehi_sb[:], in_=ehi_d[hc])
                eng2.dma_start(out=elo_sb[:], in_=elo_d[hc])
                first = hc == 0
                last = hc == HC - 1
                for bb in range(BB):
                    for lt in range(L // NL):
                        sl = slice(lt * NL, (lt + 1) * NL)
                        nc.tensor.matmul(
                            A_ps[:, sl],
                            vpad[:, hc, bb, :],
                            ehi_sb[:, bb, sl],
                            start=(first and bb == 0),
                            stop=False,
                        )
                        nc.tensor.matmul(
                            A_ps[0:BB, sl],
                            vpad[:, hc, bb, 0:BB],
                            elo_sb[:, bb, sl],
                            start=False,
                            stop=(last and bb == BB - 1),
                        )

            # E = A[0:8] + A[8:16]: DVE lanes can't cross partitions and
            # engine APs must start at a 32-aligned partition, so copy all
            # 16 rows to SBUF, DMA rows 8-15 down to partitions 0-7, add.
            a_sb = small.tile([2 * BB, L], F32)
            nc.vector.tensor_copy(a_sb[:], A_ps[:])
            hi2 = small.tile([BB, L], F32)
            nc.gpsimd.dma_start(out=hi2[:], in_=a_sb[BB : 2 * BB, :])
            E_sb = small.tile([BB, L], F32)
            nc.vector.tensor_add(E_sb[:], a_sb[0:BB, :], hi2[:])

            # softmax over l (free dim), rows are batches
            negmax = small.tile([BB, 1], F32)
            nc.vector.reduce_max(
                negmax[:], E_sb[:], axis=mybir.AxisListType.X, negate=True
            )
            p_sb = small.tile([BB, L], F32)
            esum = small.tile([BB, 1], F32)
            nc.scalar.activation(
                p_sb[:],
                E_sb[:],
                mybir.ActivationFunctionType.Exp,
                bias=negmax[:],
                scale=1.0,
                accum_out=esum[:],
            )
            rec = small.tile([BB, 1], F32)
            nc.vector.reciprocal(rec[:], esum[:])
            o_sb = small.tile([BB, L], F32)
            nc.vector.tensor_scalar_mul(o_sb[:], p_sb[:], rec[:])
            nc.sync.dma_start(out=out_d[:], in_=o_sb[:])

    nc.compile()
    return nc


def _get_nc():
    if "nc" not in _CACHE:
        _CACHE["nc"] = _build_nc()
    return _CACHE["nc"]


def _make_in_maps(hidden, enc, W):
    hidden = np.asarray(hidden, dtype=np.float32)
    enc = np.asarray(enc, dtype=np.float32)
    W = np.ascontiguousarray(np.asarray(W, dtype=np.float32))
    in_maps = []
    for c in range(N_CORES):
        sl = slice(c * BB, (c + 1) * BB)
        # [L, BB, H] -> [H, BB, L] -> [HC, P, BB, L]
        encT = np.ascontiguousarray(enc[:, sl, :].transpose(2, 1, 0)).reshape(
            HC, P, BB, L
        )
        ehi = encT.astype(BF16NP)
        elo = (encT - ehi.astype(np.float32)).astype(BF16NP)
        # [BB, H] -> [H, BB] -> [GC, P, BB] -> [P, GC, BB]
        hT = np.ascontiguousarray(
            hidden[0, sl, :].T.reshape(GC, P, BB).transpose(1, 0, 2)
        )
        in_maps.append({"ehi": ehi, "elo": elo, "w": W, "hT": hT})
    return in_maps


def kernel(hidden, encoder_outputs, W, b):
    nc = _get_nc()
    in_maps = _make_in_maps(hidden, encoder_outputs, W)
    res = run_bass_kernel_spmd(nc, in_maps, list(range(N_CORES))).results
    out = np.concatenate([res[c]["out"] for c in range(N_CORES)], axis=0)
    return out[:, None, :]


# revision 22
# speedup vs baseline: 1.3829x; 1.1488x over previous
"""Luong 'general' attention kernel for TRN2, data-parallel over batch on 8 cores.

Reference computes:
    proj[l,b,g]   = sum_h enc[l,b,h] * W[g,h] + bias[g]
    energies[b,l] = sum_g hidden[b,g] * proj[l,b,g]
    out           = softmax_l(energies)[:, None, :]

Algebraic restructure (exact):
    energies[b,l] = sum_h v[b,h] * enc[l,b,h] + c[b],   v = hidden @ W
and c[b] = hidden[b]·bias is constant over l, so it cancels in softmax.
This reduces the work from O(L*B*H*H) to O(B*H*H + L*B*H): the kernel is
bound by streaming enc (32 MB/core) from HBM, not by compute.

Precision strategy: fp32 matmuls cost 4 cycles/row on TensorE, so enc is
split on the host into bf16 hi + bf16 lo (e = e_hi + e_lo exactly to
~2^-17) and v is split on-device the same way. Energies are assembled
from three bf16 products, all PSUM-accumulated in fp32:
    rows 0-7  of A:  e_hi.v_hi + e_lo.v_hi      (two matmul streams)
    rows 8-15 of A:  e_hi.v_lo                  (packed in the e_hi stream
                                                 via a 16-col weight)
    E = A[0:8] + A[8:16]   (the dropped e_lo.v_lo term is ~2^-18 relative)
This keeps fp32-class accuracy at bf16 streaming rate (1 cycle/row).

Per-core layout (B sharded 8 ways, bb = 8 batches/core):
    ehi/elo[hc, h_in, bb, l] -- host-transposed so H is on partitions; each
                                SBUF partition row is 16 KB contiguous DRAM
    hT[g_in, gc, bb]         -- host-transposed hidden (fp32)
    W[g, h]                  -- replicated (fp32; v must be fp32-accurate)
DMA: W goes first, split across both HWDGE rings (SP + ACT), then enc
tiles alternate rings; small tensors ride the SWDGE ring.
"""

import numpy as np
import ml_dtypes

import concourse.bacc as bacc
import concourse.mybir as mybir
import concourse.tile as tile
from concourse.bass_utils import run_bass_kernel_spmd

B, L, H = 64, 1024, 1024
N_CORES = 8
BB = B // N_CORES  # batches per core
P = 128            # partitions
HC = H // P        # h chunks
GC = H // P        # g chunks
F32 = mybir.dt.float32
BF16 = mybir.dt.bfloat16
BF16NP = ml_dtypes.bfloat16

_CACHE = {}


def _build_nc():
    nc = bacc.Bacc(
        "TRN2", target_bir_lowering=False, debug=False, num_devices=N_CORES
    )

    ehi_d = nc.dram_tensor("ehi", [HC, P, BB, L], BF16, kind="ExternalInput")
    elo_d = nc.dram_tensor("elo", [HC, P, BB, L], BF16, kind="ExternalInput")
    w_d = nc.dram_tensor("w", [H, H], F32, kind="ExternalInput")
    hT_d = nc.dram_tensor("hT", [P, GC, BB], F32, kind="ExternalInput")
    id_d = nc.dram_tensor("ident", [BB, BB], F32, kind="ExternalInput")
    out_d = nc.dram_tensor("out", [BB, L], F32, kind="ExternalOutput")

    with tile.TileContext(nc) as tc:
        with (
            tc.tile_pool(name="wpool", bufs=1) as wpool,
            tc.tile_pool(name="encpool", bufs=4) as encpool,
            tc.tile_pool(name="small", bufs=1) as small,
            tc.tile_pool(name="psum", bufs=1, space="PSUM") as psum,
        ):
            hT_sb = small.tile([P, GC, BB], F32)
            nc.gpsimd.dma_start(out=hT_sb[:], in_=hT_d[:])
            idf_sb = small.tile([BB, BB], F32)
            nc.gpsimd.dma_start(out=idf_sb[:], in_=id_d[:])

            # W first, split across both HWDGE rings, so v is ready early
            w_tiles = []
            for gc in range(GC):
                wt = wpool.tile([P, H], F32, tag=f"w{gc}", name=f"w_sb{gc}")
                eng = nc.sync if gc % 2 == 0 else nc.scalar
                eng.dma_start(out=wt[:], in_=w_d[gc * P : (gc + 1) * P, :])
                w_tiles.append(wt)

            # v[bb, h] = sum_g hidden[bb,g] W[g,h]: 16 big matmuls with hT
            # stationary and W streaming (N=512), accumulated over gc
            v_ps = psum.tile([BB, H], F32)
            NL = 512  # one fp32 PSUM bank per matmul
            for lt in range(H // NL):
                for gc in range(GC):
                    nc.tensor.matmul(
                        v_ps[:, lt * NL : (lt + 1) * NL],
                        hT_sb[:, gc, :],
                        w_tiles[gc][:, lt * NL : (lt + 1) * NL],
                        start=(gc == 0),
                        stop=(gc == GC - 1),
                    )

            # PE-transpose v back to [h_in, bb] orientation per h-chunk
            # (fp32; 16-bit PSUM writes are not a TRN2 feature)
            v_sb = small.tile([BB, H], F32)
            nc.vector.tensor_copy(v_sb[:], v_ps[:])
            vT_ps = psum.tile([P, HC, BB], F32)
            for hc in range(HC):
                nc.tensor.transpose(
                    vT_ps[:, hc, :], v_sb[:, hc * P : (hc + 1) * P], idf_sb[:]
                )

            # packed diag weights: per (hc, bb) a [128, 16] block whose
            # col bb is v_hi (bf16 round of v) and col 8+bb is v_lo
            # (bf16 of v - v_hi), zeros elsewhere
            vpad = small.tile([P, HC, BB, 2 * BB], BF16)
            nc.vector.memset(vpad[:], 0.0)
            for hc in range(HC):
                blk = vpad[:, hc].rearrange("p a b -> p (a b)")
                hi_diag = blk[:, 0 : BB * 2 * BB : 2 * BB + 1]
                lo_diag = blk[:, BB : BB * 2 * BB : 2 * BB + 1]
                nc.vector.tensor_copy(hi_diag, vT_ps[:, hc, :])
                nc.vector.tensor_sub(lo_diag, vT_ps[:, hc, :], hi_diag)

            # A rows 0-7: e_hi.v_hi + e_lo.v_hi ; rows 8-15: e_hi.v_lo
            A_ps = psum.tile([2 * BB, L], F32)
            for hc in range(HC):
                ehi_sb = encpool.tile(
                    [P, BB, L], BF16, tag="ehi", name=f"ehi_sb{hc}"
                )
                elo_sb = encpool.tile(
                    [P, BB, L], BF16, tag="elo", name=f"elo_sb{hc}"
                )
                eng = nc.sync if hc % 2 == 0 else nc.scalar
                eng2 = nc.scalar if hc % 2 == 0 else nc.sync
                eng.dma_start(out=ehi_sb[:], in_=ehi_d[hc])
                eng2.dma_start(out=elo_sb[:], in_=elo_d[hc])
                first = hc == 0
                last = hc == HC - 1
                for bb in range(BB):
                    for lt in range(L // NL):
                        sl = slice(lt * NL, (lt + 1) * NL)
                        # the 16-row ehi matmul opens and closes each
                        # region's accumulation group (it covers rows 8-15
                        # that the 8-row elo matmuls never touch), so it
                        # goes last on the final iteration
                        closing = last and bb == BB - 1
                        if not closing:
                            nc.tensor.matmul(
                                A_ps[:, sl],
                                vpad[:, hc, bb, :],
                                ehi_sb[:, bb, sl],
                                start=(first and bb == 0),
                                stop=False,
                            )
                        nc.tensor.matmul(
                            A_ps[0:BB, sl],
                            vpad[:, hc, bb, 0:BB],
                            elo_sb[:, bb, sl],
                            start=False,
                            stop=False,
                        )
                        if closing:
                            nc.tensor.matmul(
                                A_ps[:, sl],
                                vpad[:, hc, bb, :],
                                ehi_sb[:, bb, sl],
                                start=False,
                                stop=True,
                            )

            # E = A[0:8] + A[8:16]: DVE lanes can't cross partitions and
            # engine APs must start at a 32-aligned partition, so copy all
            # 16 rows to SBUF, DMA rows 8-15 down to partitions 0-7, add.
            a_sb = small.tile([2 * BB, L], F32)
            nc.vector.tensor_copy(a_sb[:], A_ps[:])
            hi2 = small.tile([BB, L], F32)
            nc.gpsimd.dma_start(out=hi2[:], in_=a_sb[BB : 2 * BB, :])
            E_sb = small.tile([BB, L], F32)
            nc.vector.tensor_add(E_sb[:], a_sb[0:BB, :], hi2[:])

            # softmax over l (free dim), rows are batches
            negmax = small.tile([BB, 1], F32)
            nc.vector.reduce_max(
                negmax[:], E_sb[:], axis=mybir.AxisListType.X, negate=True
            )
            p_sb = small.tile([BB, L], F32)
            esum = small.tile([BB, 1], F32)
            nc.scalar.activation(
                p_sb[:],
                E_sb[:],
                mybir.ActivationFunctionType.Exp,
                bias=negmax[:],
                scale=1.0,
                accum_out=esum[:],
            )
            rec = small.tile([BB, 1], F32)
            nc.vector.reciprocal(rec[:], esum[:])
            o_sb = small.tile([BB, L], F32)
            nc.vector.tensor_scalar_mul(o_sb[:], p_sb[:], rec[:])
            nc.sync.dma_start(out=out_d[:], in_=o_sb[:])

    nc.compile()
    return nc


def _get_nc():
    if "nc" not in _CACHE:
        _CACHE["nc"] = _build_nc()
    return _CACHE["nc"]


def _make_in_maps(hidden, enc, W):
    hidden = np.asarray(hidden, dtype=np.float32)
    enc = np.asarray(enc, dtype=np.float32)
    W = np.ascontiguousarray(np.asarray(W, dtype=np.float32))
    in_maps = []
    for c in range(N_CORES):
        sl = slice(c * BB, (c + 1) * BB)
        # [L, BB, H] -> [H, BB, L] -> [HC, P, BB, L]
        encT = np.ascontiguousarray(enc[:, sl, :].transpose(2, 1, 0)).reshape(
            HC, P, BB, L
        )
        ehi = encT.astype(BF16NP)
        elo = (encT - ehi.astype(np.float32)).astype(BF16NP)
        # [BB, H] -> [H, BB] -> [GC, P, BB] -> [P, GC, BB]
        hT = np.ascontiguousarray(
            hidden[0, sl, :].T.reshape(GC, P, BB).transpose(1, 0, 2)
        )
        in_maps.append(
            {
                "ehi": ehi,
                "elo": elo,
                "w": W,
                "hT": hT,
                "ident": np.eye(BB, dtype=np.float32),
            }
        )
    return in_maps


def kernel(hidden, encoder_outputs, W, b):
    nc = _get_nc()
    in_maps = _make_in_maps(hidden, encoder_outputs, W)
    res = run_bass_kernel_spmd(nc, in_maps, list(range(N_CORES))).results
    out = np.concatenate([res[c]["out"] for c in range(N_CORES)], axis=0)
    return out[:, None, :]


# revision 23
# speedup vs baseline: 1.4415x; 1.0424x over previous
"""Luong 'general' attention kernel for TRN2, data-parallel over batch on 8 cores.

Reference computes:
    proj[l,b,g]   = sum_h enc[l,b,h] * W[g,h] + bias[g]
    energies[b,l] = sum_g hidden[b,g] * proj[l,b,g]
    out           = softmax_l(energies)[:, None, :]

Algebraic restructure (exact):
    energies[b,l] = sum_h v[b,h] * enc[l,b,h] + c[b],   v = hidden @ W
and c[b] = hidden[b]·bias is constant over l, so it cancels in softmax.
This reduces the work from O(L*B*H*H) to O(B*H*H + L*B*H): the kernel is
bound by streaming enc (32 MB/core) from HBM, not by compute.

Precision strategy: fp32 matmuls cost 4 cycles/row on TensorE, so enc is
split on the host into bf16 hi + bf16 lo (e = e_hi + e_lo exactly to
~2^-17) and v is split on-device the same way. Energies are assembled
from three bf16 products, all PSUM-accumulated in fp32:
    rows 0-7  of A:  e_hi.v_hi + e_lo.v_hi      (two matmul streams)
    rows 8-15 of A:  e_hi.v_lo                  (packed in the e_hi stream
                                                 via a 16-col weight)
    E = A[0:8] + A[8:16]   (the dropped e_lo.v_lo term is ~2^-18 relative)
This keeps fp32-class accuracy at bf16 streaming rate (1 cycle/row).

Per-core layout (B sharded 8 ways, bb = 8 batches/core):
    ehi/elo[hc, h_in, bb, l] -- host-transposed so H is on partitions; each
                                SBUF partition row is 16 KB contiguous DRAM
    hT[g_in, gc, bb]         -- host-transposed hidden (fp32)
    W[g, h]                  -- replicated (fp32; v must be fp32-accurate)
DMA: W goes first, split across both HWDGE rings (SP + ACT), then enc
tiles alternate rings; small tensors ride the SWDGE ring.
"""

import numpy as np
import ml_dtypes

import concourse.bacc as bacc
import concourse.mybir as mybir
import concourse.tile as tile
from concourse.bass_utils import run_bass_kernel_spmd

B, L, H = 64, 1024, 1024
N_CORES = 8
BB = B // N_CORES  # batches per core
P = 128            # partitions
HC = H // P        # h chunks
GC = H // P        # g chunks
F32 = mybir.dt.float32
BF16 = mybir.dt.bfloat16
BF16NP = ml_dtypes.bfloat16
FP8 = mybir.dt.float8e4
FP8NP = ml_dtypes.float8_e4m3
ELO_SCALE = 1024.0  # keep scaled e_lo inside fp8e4m3's normal range

_CACHE = {}


def _build_nc():
    nc = bacc.Bacc(
        "TRN2", target_bir_lowering=False, debug=False, num_devices=N_CORES
    )

    ehi_d = nc.dram_tensor("ehi", [HC, P, BB, L], BF16, kind="ExternalInput")
    elo_d = nc.dram_tensor("elo", [HC, P, BB, L], FP8, kind="ExternalInput")
    w_d = nc.dram_tensor("w", [H, H], F32, kind="ExternalInput")
    hT_d = nc.dram_tensor("hT", [P, GC, BB], F32, kind="ExternalInput")
    id_d = nc.dram_tensor("ident", [BB, BB], F32, kind="ExternalInput")
    out_d = nc.dram_tensor("out", [BB, L], F32, kind="ExternalOutput")

    with tile.TileContext(nc) as tc:
        with (
            tc.tile_pool(name="wpool", bufs=1) as wpool,
            tc.tile_pool(name="encpool", bufs=4) as encpool,
            tc.tile_pool(name="small", bufs=1) as small,
            tc.tile_pool(name="psum", bufs=1, space="PSUM") as psum,
        ):
            hT_sb = small.tile([P, GC, BB], F32)
            nc.gpsimd.dma_start(out=hT_sb[:], in_=hT_d[:])
            idf_sb = small.tile([BB, BB], F32)
            nc.gpsimd.dma_start(out=idf_sb[:], in_=id_d[:])

            # W first, split across both HWDGE rings, so v is ready early
            w_tiles = []
            for gc in range(GC):
                wt = wpool.tile([P, H], F32, tag=f"w{gc}", name=f"w_sb{gc}")
                eng = nc.sync if gc % 2 == 0 else nc.scalar
                eng.dma_start(out=wt[:], in_=w_d[gc * P : (gc + 1) * P, :])
                w_tiles.append(wt)

            # v[bb, h] = sum_g hidden[bb,g] W[g,h]: 16 big matmuls with hT
            # stationary and W streaming (N=512), accumulated over gc
            v_ps = psum.tile([BB, H], F32)
            NL = 512  # one fp32 PSUM bank per matmul
            for lt in range(H // NL):
                for gc in range(GC):
                    nc.tensor.matmul(
                        v_ps[:, lt * NL : (lt + 1) * NL],
                        hT_sb[:, gc, :],
                        w_tiles[gc][:, lt * NL : (lt + 1) * NL],
                        start=(gc == 0),
                        stop=(gc == GC - 1),
                    )

            # PE-transpose v back to [h_in, bb] orientation per h-chunk
            # (fp32; 16-bit PSUM writes are not a TRN2 feature)
            v_sb = small.tile([BB, H], F32)
            nc.vector.tensor_copy(v_sb[:], v_ps[:])
            vT_ps = psum.tile([P, HC, BB], F32)
            for hc in range(HC):
                nc.tensor.transpose(
                    vT_ps[:, hc, :], v_sb[:, hc * P : (hc + 1) * P], idf_sb[:]
                )

            # packed diag weights: per (hc, bb) a [128, 16] block whose
            # col bb is v_hi (bf16 round of v) and col 8+bb is v_lo
            # (bf16 of v - v_hi), zeros elsewhere
            vpad = small.tile([P, HC, BB, 2 * BB], BF16)
            nc.vector.memset(vpad[:], 0.0)
            for hc in range(HC):
                blk = vpad[:, hc].rearrange("p a b -> p (a b)")
                hi_diag = blk[:, 0 : BB * 2 * BB : 2 * BB + 1]
                lo_diag = blk[:, BB : BB * 2 * BB : 2 * BB + 1]
                nc.vector.tensor_copy(hi_diag, vT_ps[:, hc, :])
                nc.vector.tensor_sub(lo_diag, vT_ps[:, hc, :], hi_diag)

            # fp8 copy of the v_hi diag for the e_lo stream
            vpad8 = small.tile([P, HC, BB, BB], FP8)
            nc.vector.memset(vpad8[:], 0.0)
            for hc in range(HC):
                blk8 = vpad8[:, hc].rearrange("p a b -> p (a b)")
                nc.vector.tensor_copy(
                    blk8[:, 0 : BB * BB : BB + 1], vT_ps[:, hc, :]
                )

            # A rows 0-7: e_hi.v_hi ; rows 8-15: e_hi.v_lo
            # B rows 0-7: (1024*e_lo).v_hi  (descaled during the merge)
            A_ps = psum.tile([2 * BB, L], F32)
            B_ps = psum.tile([BB, L], F32)
            for hc in range(HC):
                ehi_sb = encpool.tile(
                    [P, BB, L], BF16, tag="ehi", name=f"ehi_sb{hc}"
                )
                elo_sb = encpool.tile(
                    [P, BB, L], FP8, tag="elo", name=f"elo_sb{hc}"
                )
                eng = nc.sync if hc % 2 == 0 else nc.scalar
                eng2 = nc.scalar if hc % 2 == 0 else nc.sync
                eng.dma_start(out=ehi_sb[:], in_=ehi_d[hc])
                eng2.dma_start(out=elo_sb[:], in_=elo_d[hc])
                first = hc == 0
                last = hc == HC - 1
                for bb in range(BB):
                    for lt in range(L // NL):
                        sl = slice(lt * NL, (lt + 1) * NL)
                        nc.tensor.matmul(
                            A_ps[:, sl],
                            vpad[:, hc, bb, :],
                            ehi_sb[:, bb, sl],
                            start=(first and bb == 0),
                            stop=(last and bb == BB - 1),
                        )
                        nc.tensor.matmul(
                            B_ps[:, sl],
                            vpad8[:, hc, bb, :],
                            elo_sb[:, bb, sl],
                            start=(first and bb == 0),
                            stop=(last and bb == BB - 1),
                        )

            # E = A[0:8] + A[8:16]: DVE lanes can't cross partitions and
            # engine APs must start at a 32-aligned partition, so copy all
            # 16 rows to SBUF, DMA rows 8-15 down to partitions 0-7, add.
            a_sb = small.tile([2 * BB, L], F32)
            nc.vector.tensor_copy(a_sb[:], A_ps[:])
            hi2 = small.tile([BB, L], F32)
            nc.gpsimd.dma_start(out=hi2[:], in_=a_sb[BB : 2 * BB, :])
            e1_sb = small.tile([BB, L], F32)
            nc.vector.scalar_tensor_tensor(
                e1_sb[:],
                B_ps[:],
                1.0 / ELO_SCALE,
                hi2[:],
                op0=mybir.AluOpType.mult,
                op1=mybir.AluOpType.add,
            )
            E_sb = small.tile([BB, L], F32)
            nc.vector.tensor_add(E_sb[:], a_sb[0:BB, :], e1_sb[:])

            # softmax over l (free dim), rows are batches
            negmax = small.tile([BB, 1], F32)
            nc.vector.reduce_max(
                negmax[:], E_sb[:], axis=mybir.AxisListType.X, negate=True
            )
            p_sb = small.tile([BB, L], F32)
            esum = small.tile([BB, 1], F32)
            nc.scalar.activation(
                p_sb[:],
                E_sb[:],
                mybir.ActivationFunctionType.Exp,
                bias=negmax[:],
                scale=1.0,
                accum_out=esum[:],
            )
            rec = small.tile([BB, 1], F32)
            nc.vector.reciprocal(rec[:], esum[:])
            o_sb = small.tile([BB, L], F32)
            nc.vector.tensor_scalar_mul(o_sb[:], p_sb[:], rec[:])
            nc.sync.dma_start(out=out_d[:], in_=o_sb[:])

    nc.compile()
    return nc


def _get_nc():
    if "nc" not in _CACHE:
        _CACHE["nc"] = _build_nc()
    return _CACHE["nc"]


def _make_in_maps(hidden, enc, W):
    hidden = np.asarray(hidden, dtype=np.float32)
    enc = np.asarray(enc, dtype=np.float32)
    W = np.ascontiguousarray(np.asarray(W, dtype=np.float32))
    in_maps = []
    for c in range(N_CORES):
        sl = slice(c * BB, (c + 1) * BB)
        # [L, BB, H] -> [H, BB, L] -> [HC, P, BB, L]
        encT = np.ascontiguousarray(enc[:, sl, :].transpose(2, 1, 0)).reshape(
            HC, P, BB, L
        )
        ehi = encT.astype(BF16NP)
        elo = ((encT - ehi.astype(np.float32)) * ELO_SCALE).astype(FP8NP)
        # [BB, H] -> [H, BB] -> [GC, P, BB] -> [P, GC, BB]
        hT = np.ascontiguousarray(
            hidden[0, sl, :].T.reshape(GC, P, BB).transpose(1, 0, 2)
        )
        in_maps.append(
            {
                "ehi": ehi,
                "elo": elo,
                "w": W,
                "hT": hT,
                "ident": np.eye(BB, dtype=np.float32),
            }
        )
    return in_maps


def kernel(hidden, encoder_outputs, W, b):
    nc = _get_nc()
    in_maps = _make_in_maps(hidden, encoder_outputs, W)
    res = run_bass_kernel_spmd(nc, in_maps, list(range(N_CORES))).results
    out = np.concatenate([res[c]["out"] for c in range(N_CORES)], axis=0)
    return out[:, None, :]
